# revision 8
# baseline (speedup 1.0000x reference)
"""Trainium2 Bass kernel for nn_Grid_fun: out = tile(feat(z), 6) @ a.

Math: z = [x, 1] (N,4); feat = (z (x) z).reshape(N,16); out = tile(feat,6) @ a
    = feat @ a_eff  where a_eff = a.reshape(6,16,3).sum(0)   [16,3]
    => out[n,c] = z[n]^T A_c z[n],  A_c = a_eff[:,c].reshape(4,4) symmetrized.

Key trick (host, per-call): solve  sum_s W[s,c] * v_s v_s^T = A_c  exactly with
s=6 rank-1 squares shared across the 3 channels (LM on 24 vars; fallbacks s=7,
s=9 use the same device program with more columns).  Then per point:
    out_c = sum_s W[s,c] * (v_s . z)^2
Device pipeline per 512-column chunk (G=21 points/z-column, 6 forms/point,
126 = 6*21 forms per column):
    mm1 (PE, K=64, M=126, dual row-groups h0/h64): V = pv^T @ z
    square (ACT Square for half A, DVE tensor_mul for half B): R = V^2 -> fp16
    mm2 (PE, K=126, M=64, dual col-groups h0/h64):  out = ab^T @ R
    evac (ACT copy / DVE copy alternating):         psum -> sbuf fp16 -> DMA
No PE warm-up: in this environment the PE stays at 1.2 GHz regardless (all 8
cores active), so warm-up matmuls are pure overhead.
"""

import sys

if "/opt/trn_rl_repo" not in sys.path:
    sys.path.insert(0, "/opt/trn_rl_repo")

from contextlib import ExitStack

import numpy as np

import concourse.bass as bass
import concourse.mybir as mybir
import concourse.tile as tile
from concourse import bacc
from concourse.bass_utils import run_bass_kernel_spmd

N_CORES = 8
N_POINTS = 1_000_000
N_PER_CORE = N_POINTS // N_CORES  # 125000
M1 = 126  # forms per z-column = s * G
CH = 512
NHALF = 64512  # M1/s * cols = G * cols, points per half (all s variants)

_CACHE: dict = {}


# ----------------------------------------------------------------- device ---
def _build_nc(cols: int):
    n_sc = cols // CH
    n_pairs = n_sc // 2
    assert n_sc % 2 == 0
    nc = bacc.Bacc("TRN2", target_bir_lowering=False)
    f32 = mybir.dt.float32
    f16 = mybir.dt.float16

    z_d = nc.dram_tensor("z", [128, cols], f16, kind="ExternalInput")
    pv_d = nc.dram_tensor("pv", [128, M1], f16, kind="ExternalInput")
    ab_d = nc.dram_tensor("ab", [128, 64], f16, kind="ExternalInput")
    o_d = nc.dram_tensor("o", [128, cols], f16, kind="ExternalOutput")
    sink_d = nc.dram_tensor("sink", [128, 2], f32, kind="ExternalOutput")

    with tile.TileContext(nc) as tc:
        with ExitStack() as ctx:
            cpool = ctx.enter_context(tc.tile_pool(name="consts", bufs=1))
            zpool = ctx.enter_context(tc.tile_pool(name="zp", bufs=1))
            rpool = ctx.enter_context(tc.tile_pool(name="rp", bufs=3))
            opool = ctx.enter_context(tc.tile_pool(name="op", bufs=3))
            vpool = ctx.enter_context(
                tc.tile_pool(name="vp", bufs=3, space="PSUM")
            )
            ppool = ctx.enter_context(
                tc.tile_pool(name="pp", bufs=2, space="PSUM")
            )

            # ACT Square table load off the critical path: dummy square on a
            # memset tile (no DMA dep) makes walrus put the table load first
            # on the scalar queue; result sunk (DMA'd at the very end so the
            # sync queue never stalls on it).
            wt = cpool.tile([128, 2], f32)
            nc.gpsimd.memset(wt[:], 1.0)
            wsq = cpool.tile([128, 2], f32)
            nc.scalar.square(wsq[:, 0:1], wt[:, 0:1])
            nc.vector.tensor_copy(wsq[:, 1:2], wt[:, 1:2])

            # input DMAs: ALL on the sync queue (hardware DGE; the gpsimd
            # queue is the slow software DGE). pv/ab first: they gate the
            # first LDWEIGHTS and are tiny.
            pv = cpool.tile([128, M1], f16)
            nc.sync.dma_start(pv[:], pv_d[:, :])
            ab = cpool.tile([128, 64], f16)
            nc.sync.dma_start(ab[:], ab_d[:, :])
            zts = []
            for q in range(n_pairs):
                zt = zpool.tile([128, 2 * CH], f16, name=f"zt{q}")
                nc.sync.dma_start(
                    zt[:], z_d[:, 2 * CH * q : 2 * CH * (q + 1)]
                )
                zts.append(zt)

            rtiles = {}

            def mm2_pair(q):
                for v in (2 * q, 2 * q + 1):
                    rt = rtiles.pop(v)
                    po = ppool.tile([128, CH], f32, name="po")
                    nc.tensor.matmul(
                        po[0:64, :], ab[0:M1, :], rt[0:M1, 0:CH],
                        start=True, stop=True,
                    )
                    nc.tensor.matmul(
                        po[64:128, :], ab[0:M1, :], rt[0:M1, CH : 2 * CH],
                        start=True, stop=True,
                    )
                    ot = opool.tile([128, CH], f16, name="ot")
                    nc.vector.tensor_copy(ot[:], po[:])
                    nc.sync.dma_start(o_d[:, CH * v : CH * (v + 1)], ot[:])

            for p in range(n_pairs):
                for v in (2 * p, 2 * p + 1):
                    off = CH * (v % 2)
                    zt = zts[p]
                    # V for both halves in one 2-bank PSUM tile: cols
                    # 0:512 = half A forms, 512:1024 = half B forms.
                    vt = vpool.tile([M1, 2 * CH], f32, name="vt")
                    nc.tensor.matmul(
                        vt[:, 0:CH], pv[0:64, :], zt[0:64, off : off + CH],
                        start=True, stop=True,
                    )
                    nc.tensor.matmul(
                        vt[:, CH : 2 * CH], pv[64:128, :],
                        zt[64:128, off : off + CH],
                        start=True, stop=True,
                    )
                    rt = rpool.tile([128, 2 * CH], f16, name="rt")
                    nc.scalar.square(rt[0:M1, :], vt[:])
                    rtiles[v] = rt
                if p > 0:
                    mm2_pair(p - 1)
            mm2_pair(n_pairs - 1)
            nc.sync.dma_start(sink_d[:, :], wsq[:])
    nc.compile()
    return nc


# ------------------------------------------------------------ host solver ---
_SYM_IDX = [(0, 0), (1, 1), (2, 2), (3, 3), (0, 1), (0, 2), (0, 3),
            (1, 2), (1, 3), (2, 3)]


def _sym10(M):
    r2 = np.sqrt(2.0)
    return np.array(
        [M[i, j] * (1.0 if i == j else r2) for i, j in _SYM_IDX]
    )


def _residual(V, T):
    S = np.stack(
        [_sym10(np.outer(V[:, s], V[:, s])) for s in range(V.shape[1])],
        axis=1,
    )
    W, *_ = np.linalg.lstsq(S, T, rcond=None)
    return (S @ W - T).ravel(), W


def _solve_forms(As, s, rng, n_restarts=24, iters=250, tol=1e-18):
    """Find V [4,s], W [s,3] with sum_s W[s,c] v_s v_s^T = As[c] (exact)."""
    T = np.stack([_sym10(As[c]) for c in range(3)], axis=1)  # [10,3]
    scale = max(np.abs(T).max(), 1e-6)
    best = None
    nv = 4 * s
    for _ in range(n_restarts):
        V = rng.standard_normal((4, s))
        V /= np.linalg.norm(V, axis=0, keepdims=True)
        lam = 1e-3
        res, W = _residual(V, T)
        f = res @ res
        for _ in range(iters):
            J = np.zeros((res.size, nv))
            eps = 1e-6
            for k in range(nv):
                Vp = V.ravel().copy()
                Vp[k] += eps
                rp, _ = _residual(Vp.reshape(4, s), T)
                J[:, k] = (rp - res) / eps
            try:
                d = np.linalg.solve(J.T @ J + lam * np.eye(nv), J.T @ res)
            except np.linalg.LinAlgError:
                break
            Vn = (V.ravel() - d).reshape(4, s)
            rn, Wn = _residual(Vn, T)
            fn = rn @ rn
            if fn < f:
                V, res, W, f = Vn, rn, Wn, fn
                lam = max(lam * 0.5, 1e-9)
                if f < tol * scale * scale:
                    break
            else:
                lam *= 3.0
                if lam > 1e7:
                    break
        if f < 1e-14 * scale * scale:
            cand = (np.abs(W).max(), V.copy(), W.copy())
            if best is None or cand[0] < best[0]:
                best = cand
            if best[0] < 25.0:
                break
    return best  # (wmax, V, W) or None


def _forms_v9(As):
    """Deterministic 9-form construction (always works; tiny const error)."""
    Q = As[:, :3, :3]
    L = 2.0 * As[:, :3, 3]
    K = As[:, 3, 3].copy()
    Ksafe = np.where(np.abs(K) < 1e-3, 1.0, K)
    U = L / (2.0 * Ksafe[:, None])
    E = np.eye(3)
    dirs = [E[0], E[1], E[2], E[0] + E[1], E[0] + E[2], E[1] + E[2]]
    V = np.zeros((4, 9))
    for si, u in enumerate(dirs):
        V[:3, si] = u
    for c in range(3):
        V[:3, 6 + c] = U[c]
        V[3, 6 + c] = 1.0
    # solve W by lstsq on the sym10 embedding (kconst residual ~<=1e-3 abs)
    T = np.stack([_sym10(As[c]) for c in range(3)], axis=1)
    S = np.stack(
        [_sym10(np.outer(V[:, s], V[:, s])) for s in range(9)], axis=1
    )
    W, *_ = np.linalg.lstsq(S, T, rcond=None)
    return V, W


def _decompose(a: np.ndarray):
    """a [96,3] -> (s, V [4,s], W [s,3])."""
    a_eff = a.astype(np.float64).reshape(6, 16, 3).sum(0)
    A = a_eff.T.reshape(3, 4, 4)
    As = 0.5 * (A + A.transpose(0, 2, 1))
    rng = np.random.default_rng(12345)
    for s, nr in ((6, 24), (7, 16), (9, 16)):
        got = _solve_forms(As, s, rng, n_restarts=nr)
        if got is not None:
            return s, got[1], got[2]
    V, W = _forms_v9(As)
    return 9, V, W


# --------------------------------------------------------------- packing ---
def _host_tensors(V: np.ndarray, W: np.ndarray):
    """V [4,s], W [s,3] -> pv [128,126] f16, ab [128,64] f16."""
    s = V.shape[1]
    G = M1 // s
    pv1 = np.zeros((64, M1), dtype=np.float64)
    ab = np.zeros((128, 64), dtype=np.float64)
    for u in range(G):
        for si in range(s):
            col = s * u + si
            for j in range(3):
                pv1[3 * u + j, col] = V[j, si]
            pv1[63, col] = V[3, si]  # ones-row carries the z4 component
            ab[col, 3 * u : 3 * u + 3] = W[si, :]
    pv = np.zeros((128, M1), dtype=np.float64)
    pv[0:64] = pv1
    pv[64:128] = pv1
    return pv.astype(np.float16), ab.astype(np.float16)


def _pack_x(x_core: np.ndarray, G: int, cols: int) -> np.ndarray:
    """[N_PER_CORE,3] f32 -> z [128, cols] f16; half h rows 64h+3u+j,
    ones at rows 63/127. point p = h*NHALF + c*G + u."""
    xp = np.zeros((2 * NHALF, 3), dtype=np.float32)
    xp[:N_PER_CORE] = x_core
    z = np.zeros((128, cols), dtype=np.float32)
    for h in range(2):
        blk = xp[h * NHALF : (h + 1) * NHALF].reshape(cols, G, 3)
        z[64 * h : 64 * h + 3 * G] = blk.transpose(1, 2, 0).reshape(
            3 * G, cols
        )
        z[64 * h + 63] = 1.0
    return np.ascontiguousarray(z.astype(np.float16))


def _unpack_o(o: np.ndarray, G: int, cols: int) -> np.ndarray:
    """o [128, cols] f16 -> [N_PER_CORE, 3] f32. row 64h+3t+cc, col c
    holds channel cc of point h*NHALF + c*G + t."""
    of = np.asarray(o, dtype=np.float32)
    full = np.empty((2 * NHALF, 3), dtype=np.float32)
    for h in range(2):
        blk = of[64 * h : 64 * h + 3 * G]  # [3G, cols]
        full[h * NHALF : (h + 1) * NHALF] = (
            blk.reshape(G, 3, cols).transpose(2, 0, 1).reshape(NHALF, 3)
        )
    return full[:N_PER_CORE]


def kernel(x: np.ndarray, a: np.ndarray) -> np.ndarray:
    x = np.ascontiguousarray(x, dtype=np.float32)
    a = np.ascontiguousarray(a, dtype=np.float32)

    s, V, W = _decompose(a)
    G = M1 // s
    cols = 2 * NHALF // (2 * G)  # NHALF / G
    if cols not in _CACHE:
        _CACHE[cols] = _build_nc(cols)
    nc = _CACHE[cols]

    pv, ab = _host_tensors(V, W)
    in_maps = []
    for ci in range(N_CORES):
        z = _pack_x(x[ci * N_PER_CORE : (ci + 1) * N_PER_CORE], G, cols)
        in_maps.append({"z": z, "pv": pv, "ab": ab})

    res = run_bass_kernel_spmd(nc, in_maps, list(range(N_CORES)))

    out = np.empty((N_POINTS, 3), dtype=np.float32)
    for ci in range(N_CORES):
        out[ci * N_PER_CORE : (ci + 1) * N_PER_CORE] = _unpack_o(
            res.results[ci]["o"], G, cols
        )
    return out


# revision 16
# speedup vs baseline: 1.0309x; 1.0309x over previous
"""Trainium2 Bass kernel for nn_Grid_fun: out = tile(feat(z), 6) @ a.

Math: z = [x, 1] (N,4); feat = (z (x) z).reshape(N,16); out = tile(feat,6) @ a
    = feat @ a_eff  where a_eff = a.reshape(6,16,3).sum(0)   [16,3]
    => out[n,c] = z[n]^T A_c z[n],  A_c = a_eff[:,c].reshape(4,4) symmetrized.

Key trick (host, per-call): solve  sum_s W[s,c] * v_s v_s^T = A_c  exactly with
s=6 rank-1 squares shared across the 3 channels (LM on 24 vars; fallbacks s=7,
s=9 use the same device program with more columns).  Then per point:
    out_c = sum_s W[s,c] * (v_s . z)^2
Device pipeline per 512-column chunk (G=21 points/z-column, 6 forms/point,
126 = 6*21 forms per column):
    mm1 (PE, K=64, M=126, dual row-groups h0/h64): V = pv^T @ z
    square (ACT Square for half A, DVE tensor_mul for half B): R = V^2 -> fp16
    mm2 (PE, K=126, M=64, dual col-groups h0/h64):  out = ab^T @ R
    evac (ACT copy / DVE copy alternating):         psum -> sbuf fp16 -> DMA
No PE warm-up: in this environment the PE stays at 1.2 GHz regardless (all 8
cores active), so warm-up matmuls are pure overhead.
"""

import sys

if "/opt/trn_rl_repo" not in sys.path:
    sys.path.insert(0, "/opt/trn_rl_repo")

from contextlib import ExitStack

import numpy as np

import concourse.bass as bass
import concourse.mybir as mybir
import concourse.tile as tile
from concourse import bacc
from concourse.bass_utils import run_bass_kernel_spmd

N_CORES = 8
N_POINTS = 1_000_000
N_PER_CORE = N_POINTS // N_CORES  # 125000
M1 = 126  # forms per z-column = s * G
CH = 512
NHALF = 64512  # M1/s * cols = G * cols, points per half (all s variants)

_CACHE: dict = {}


# ----------------------------------------------------------------- device ---
def _build_nc(cols: int):
    n_sc = cols // CH
    n_pairs = n_sc // 2
    assert n_sc % 2 == 0
    nc = bacc.Bacc("TRN2", target_bir_lowering=False)
    f32 = mybir.dt.float32
    f16 = mybir.dt.float16

    z_d = nc.dram_tensor("z", [128, cols], f16, kind="ExternalInput")
    wab_d = nc.dram_tensor("wab", [128, 192], f16, kind="ExternalInput")
    o_d = nc.dram_tensor("o", [128, cols], f16, kind="ExternalOutput")
    sink_d = nc.dram_tensor("sink", [128, 2], f32, kind="ExternalOutput")

    with tile.TileContext(nc) as tc:
        with ExitStack() as ctx:
            cpool = ctx.enter_context(tc.tile_pool(name="consts", bufs=1))
            zpool = ctx.enter_context(tc.tile_pool(name="zp", bufs=1))
            rpool = ctx.enter_context(tc.tile_pool(name="rp", bufs=3))
            opool = ctx.enter_context(tc.tile_pool(name="op", bufs=3))
            vpool = ctx.enter_context(
                tc.tile_pool(name="vp", bufs=3, space="PSUM")
            )
            ppool = ctx.enter_context(
                tc.tile_pool(name="pp", bufs=2, space="PSUM")
            )

            # ACT Square table load off the critical path: dummy square on a
            # memset tile (no DMA dep) makes walrus put the table load first
            # on the scalar queue; result sunk (DMA'd at the very end so the
            # sync queue never stalls on it).
            wt = cpool.tile([128, 2], f32)
            nc.gpsimd.memset(wt[:], 1.0)
            wsq = cpool.tile([128, 2], f32)
            nc.scalar.square(wsq[:, 0:1], wt[:, 0:1])
            nc.vector.tensor_copy(wsq[:, 1:2], wt[:, 1:2])

            # input DMAs: ALL on the sync queue (hardware DGE; the gpsimd
            # queue is the slow software DGE). Order: first 512-col z chunk
            # (gates the first matmul), the combined pv+ab weights, then the
            # remaining z. DMA latency is ~1.5-2.4us from issue start.
            zt0 = zpool.tile([128, 2 * CH], f16, name="zt0")
            nc.sync.dma_start(zt0[:, 0:CH], z_d[:, 0:CH])
            wab = cpool.tile([128, 192], f16)
            nc.sync.dma_start(wab[:], wab_d[:, :])
            nc.sync.dma_start(zt0[:, CH : 2 * CH], z_d[:, CH : 2 * CH])
            zts = [zt0]
            for q in range(1, n_pairs):
                zt = zpool.tile([128, 2 * CH], f16, name=f"zt{q}")
                nc.sync.dma_start(
                    zt[:], z_d[:, 2 * CH * q : 2 * CH * (q + 1)]
                )
                zts.append(zt)


            rtiles = {}

            def mm2_pair(q, last=False):
                for v in (2 * q, 2 * q + 1):
                    rt = rtiles.pop(v)
                    po = ppool.tile([128, CH], f32, name="po")
                    nc.tensor.matmul(
                        po[0:64, :], wab[0:M1, M1 : M1 + 64], rt[0:M1, 0:CH],
                        start=True, stop=True,
                    )
                    nc.tensor.matmul(
                        po[64:128, :], wab[0:M1, M1 : M1 + 64],
                        rt[0:M1, CH : 2 * CH],
                        start=True, stop=True,
                    )
                    ot = opool.tile([128, CH], f16, name="ot")
                    # the very last evacuation goes on ACT (idle after the
                    # final square; DVE is still busy with the prior cast)
                    if last and v == 2 * q + 1:
                        nc.scalar.copy(ot[:], po[:])
                    else:
                        nc.vector.tensor_copy(ot[:], po[:])
                    nc.sync.dma_start(o_d[:, CH * v : CH * (v + 1)], ot[:])

            for p in range(n_pairs):
                for v in (2 * p, 2 * p + 1):
                    off = CH * (v % 2)
                    zt = zts[p]
                    # V for both halves in one 2-bank PSUM tile: cols
                    # 0:512 = half A forms, 512:1024 = half B forms.
                    vt = vpool.tile([M1, 2 * CH], f32, name="vt")
                    nc.tensor.matmul(
                        vt[:, 0:CH], wab[0:64, 0:M1],
                        zt[0:64, off : off + CH],
                        start=True, stop=True,
                    )
                    nc.tensor.matmul(
                        vt[:, CH : 2 * CH], wab[64:128, 0:M1],
                        zt[64:128, off : off + CH],
                        start=True, stop=True,
                    )
                    rt = rpool.tile([128, 2 * CH], f16, name="rt")
                    nc.scalar.square(rt[0:M1, :], vt[:])
                    rtiles[v] = rt
                if p > 0:
                    mm2_pair(p - 1)
            mm2_pair(n_pairs - 1, last=True)
            nc.sync.dma_start(sink_d[:, :], wsq[:])
    nc.compile()
    return nc


# ------------------------------------------------------------ host solver ---
_SYM_IDX = [(0, 0), (1, 1), (2, 2), (3, 3), (0, 1), (0, 2), (0, 3),
            (1, 2), (1, 3), (2, 3)]


def _sym10(M):
    r2 = np.sqrt(2.0)
    return np.array(
        [M[i, j] * (1.0 if i == j else r2) for i, j in _SYM_IDX]
    )


def _residual(V, T):
    S = np.stack(
        [_sym10(np.outer(V[:, s], V[:, s])) for s in range(V.shape[1])],
        axis=1,
    )
    W, *_ = np.linalg.lstsq(S, T, rcond=None)
    return (S @ W - T).ravel(), W


def _solve_forms(As, s, rng, n_restarts=24, iters=250, tol=1e-18):
    """Find V [4,s], W [s,3] with sum_s W[s,c] v_s v_s^T = As[c] (exact)."""
    T = np.stack([_sym10(As[c]) for c in range(3)], axis=1)  # [10,3]
    scale = max(np.abs(T).max(), 1e-6)
    best = None
    nv = 4 * s
    for _ in range(n_restarts):
        V = rng.standard_normal((4, s))
        V /= np.linalg.norm(V, axis=0, keepdims=True)
        lam = 1e-3
        res, W = _residual(V, T)
        f = res @ res
        for _ in range(iters):
            J = np.zeros((res.size, nv))
            eps = 1e-6
            for k in range(nv):
                Vp = V.ravel().copy()
                Vp[k] += eps
                rp, _ = _residual(Vp.reshape(4, s), T)
                J[:, k] = (rp - res) / eps
            try:
                d = np.linalg.solve(J.T @ J + lam * np.eye(nv), J.T @ res)
            except np.linalg.LinAlgError:
                break
            Vn = (V.ravel() - d).reshape(4, s)
            rn, Wn = _residual(Vn, T)
            fn = rn @ rn
            if fn < f:
                V, res, W, f = Vn, rn, Wn, fn
                lam = max(lam * 0.5, 1e-9)
                if f < tol * scale * scale:
                    break
            else:
                lam *= 3.0
                if lam > 1e7:
                    break
        if f < 1e-14 * scale * scale:
            cand = (np.abs(W).max(), V.copy(), W.copy())
            if best is None or cand[0] < best[0]:
                best = cand
            if best[0] < 25.0:
                break
    return best  # (wmax, V, W) or None


def _forms_v9(As):
    """Deterministic 9-form construction (always works; tiny const error)."""
    Q = As[:, :3, :3]
    L = 2.0 * As[:, :3, 3]
    K = As[:, 3, 3].copy()
    Ksafe = np.where(np.abs(K) < 1e-3, 1.0, K)
    U = L / (2.0 * Ksafe[:, None])
    E = np.eye(3)
    dirs = [E[0], E[1], E[2], E[0] + E[1], E[0] + E[2], E[1] + E[2]]
    V = np.zeros((4, 9))
    for si, u in enumerate(dirs):
        V[:3, si] = u
    for c in range(3):
        V[:3, 6 + c] = U[c]
        V[3, 6 + c] = 1.0
    # solve W by lstsq on the sym10 embedding (kconst residual ~<=1e-3 abs)
    T = np.stack([_sym10(As[c]) for c in range(3)], axis=1)
    S = np.stack(
        [_sym10(np.outer(V[:, s], V[:, s])) for s in range(9)], axis=1
    )
    W, *_ = np.linalg.lstsq(S, T, rcond=None)
    return V, W


def _decompose(a: np.ndarray):
    """a [96,3] -> (s, V [4,s], W [s,3])."""
    a_eff = a.astype(np.float64).reshape(6, 16, 3).sum(0)
    A = a_eff.T.reshape(3, 4, 4)
    As = 0.5 * (A + A.transpose(0, 2, 1))
    rng = np.random.default_rng(12345)
    for s, nr in ((6, 24), (7, 16), (9, 16)):
        got = _solve_forms(As, s, rng, n_restarts=nr)
        if got is not None:
            return s, got[1], got[2]
    V, W = _forms_v9(As)
    return 9, V, W


# --------------------------------------------------------------- packing ---
def _host_tensors(V: np.ndarray, W: np.ndarray):
    """V [4,s], W [s,3] -> wab [128,192] f16 (cols 0:126 pv, 126:190 ab)."""
    s = V.shape[1]
    G = M1 // s
    pv1 = np.zeros((64, M1), dtype=np.float64)
    ab = np.zeros((128, 64), dtype=np.float64)
    for u in range(G):
        for si in range(s):
            col = s * u + si
            for j in range(3):
                pv1[3 * u + j, col] = V[j, si]
            pv1[63, col] = V[3, si]  # ones-row carries the z4 component
            ab[col, 3 * u : 3 * u + 3] = W[si, :]
    wab = np.zeros((128, 192), dtype=np.float64)
    wab[0:64, 0:M1] = pv1
    wab[64:128, 0:M1] = pv1
    wab[:, M1 : M1 + 64] = ab
    return wab.astype(np.float16)


def _pack_x(x_core: np.ndarray, G: int, cols: int) -> np.ndarray:
    """[N_PER_CORE,3] f32 -> z [128, cols] f16; half h rows 64h+3u+j,
    ones at rows 63/127. point p = h*NHALF + c*G + u."""
    xp = np.zeros((2 * NHALF, 3), dtype=np.float32)
    xp[:N_PER_CORE] = x_core
    z = np.zeros((128, cols), dtype=np.float32)
    for h in range(2):
        blk = xp[h * NHALF : (h + 1) * NHALF].reshape(cols, G, 3)
        z[64 * h : 64 * h + 3 * G] = blk.transpose(1, 2, 0).reshape(
            3 * G, cols
        )
        z[64 * h + 63] = 1.0
    return np.ascontiguousarray(z.astype(np.float16))


def _unpack_o(o: np.ndarray, G: int, cols: int) -> np.ndarray:
    """o [128, cols] f16 -> [N_PER_CORE, 3] f32. row 64h+3t+cc, col c
    holds channel cc of point h*NHALF + c*G + t."""
    of = np.asarray(o, dtype=np.float32)
    full = np.empty((2 * NHALF, 3), dtype=np.float32)
    for h in range(2):
        blk = of[64 * h : 64 * h + 3 * G]  # [3G, cols]
        full[h * NHALF : (h + 1) * NHALF] = (
            blk.reshape(G, 3, cols).transpose(2, 0, 1).reshape(NHALF, 3)
        )
    return full[:N_PER_CORE]


def kernel(x: np.ndarray, a: np.ndarray) -> np.ndarray:
    x = np.ascontiguousarray(x, dtype=np.float32)
    a = np.ascontiguousarray(a, dtype=np.float32)

    s, V, W = _decompose(a)
    G = M1 // s
    cols = 2 * NHALF // (2 * G)  # NHALF / G
    if cols not in _CACHE:
        _CACHE[cols] = _build_nc(cols)
    nc = _CACHE[cols]

    wab = _host_tensors(V, W)
    in_maps = []
    for ci in range(N_CORES):
        z = _pack_x(x[ci * N_PER_CORE : (ci + 1) * N_PER_CORE], G, cols)
        in_maps.append({"z": z, "wab": wab})

    res = run_bass_kernel_spmd(nc, in_maps, list(range(N_CORES)))

    out = np.empty((N_POINTS, 3), dtype=np.float32)
    for ci in range(N_CORES):
        out[ci * N_PER_CORE : (ci + 1) * N_PER_CORE] = _unpack_o(
            res.results[ci]["o"], G, cols
        )
    return out
